# revision 6
# baseline (speedup 1.0000x reference)
"""DetectionLoss Trainium2 kernel (8 NeuronCores, pure data parallel over B).

Reference computation (see problem statement):
  - rasterize N=16 boxes per batch to per-pixel target label / target box /
    valid mask (host, numpy -- tiny work on tiny inputs)
  - focal classification loss over cls_scores (B,A,C,H,W) = (8,9,4,256,256)
  - masked SmoothL1 box loss over bbox_preds  (B,A,4,H,W)
  - scalar means -> (total, cls_loss, box_loss)

Device strategy (one batch element per core):

cls path, class-on-partition layout: partition = (c, g) with c in [0,4),
g in [0,32); pixel = g*2048 + j, j in [0,2048):
  per anchor a: e = exp(x) [ACT f32->bf16]; he = e * onehot [DVE bf16 2x];
  S = sum_c e and Et = sum_c he via TensorE delta-matmuls (lhsT[128,32] with
  W[(c,g), g'] = (g==g'), col-tiled 4 anchors per PSUM [128,2048] group);
  then per 4-anchor group: logS = ln(S), logEt = ln(Et) [ACT from PSUM];
  ce = logS - logEt; pt = exp(-ce); ace = alpha_t * ce;
  focal accum custom DVE op: sum (1-pt)^2 * ace.
  (log(Et) == target logit since Et = exp(x_target) exactly: the one-hot
   multiply zeroes the other classes and the sum adds three exact zeros.)

box path, pixel-on-partition layout (pixel = k*512 + j):
  one fused custom DVE op per anchor:
    accum += sum relu(|p-w|)^2 - relu(|p-w|-1)^2  == 2 * smoothl1(p-t) * valid
  where w = target-box-or-NaN; relu(NaN)=0 on DVE scrubs invalid pixels.

host: final scalar reductions over the tiny per-partition accumulators.
"""

import os
import sys

sys.path.insert(0, "/opt/trn_rl_repo")

from contextlib import ExitStack
from operator import add as _op_add

import ml_dtypes
import numpy as np

import concourse.bacc as bacc
import concourse.tile as tile
from concourse import mybir
from concourse.bass_utils import run_bass_kernel_spmd
from concourse.dve_spec import AluOp, Bin, C0, C1, One, Spec, Src0, Src1, lower, relu, sq
from concourse.dve_uop import DveOpSpec
import concourse.dve_ops as dvo

BF16 = mybir.dt.bfloat16
F32 = mybir.dt.float32

GAMMA = 2.0
B, A, C, H, W, N = 8, 9, 4, 256, 256, 16
HW = H * W  # 65536
PARTS = 128
G = 32  # pixel groups on partitions (cls layout)
FJ = HW // G  # 2048 free positions per group (cls layout)
FREE = HW // PARTS  # 512 (box layout)
GROUPS = [(0, 4), (4, 8), (8, 9)]

# ---------------------------------------------------------------------------
# custom DVE ops
# ---------------------------------------------------------------------------


def _dve_relu(x):
    # DVE MAX semantics: max(NaN, 0) = 0 (numpy max propagates NaN)
    return np.maximum(np.nan_to_num(x, nan=0.0, posinf=np.inf, neginf=-np.inf), 0)


def _as_col(v, P):
    a = np.asarray(v, np.float32)
    return a.reshape(-1, 1) if a.ndim else np.full((P, 1), float(a), np.float32)


def _ref_sl1(in0, in1, s0, s1, imm2):
    P = in0.shape[0]
    a = np.abs(in0.astype(np.float32) - in1.astype(np.float32))
    body = _dve_relu(a) ** 2 - _dve_relu(a - _as_col(s0, P)) ** 2
    acc = _as_col(s1, P) + body.reshape(P, -1).sum(axis=-1, keepdims=True)
    return body.astype(np.float32), acc


def _ref_ft(in0, in1, s0, s1, imm2):
    P = in0.shape[0]
    body = (1.0 - in0.astype(np.float32)) ** 2 * in1.astype(np.float32)
    acc = _as_col(s0, P) + body.reshape(P, -1).sum(axis=-1, keepdims=True)
    return body.astype(np.float32), acc


def _register(name, spec):
    for op in dvo.OPS:
        if op.name == name:  # idempotent across re-imports
            return op
    op = dvo.DveOp(name, spec, subdim=False, uops_sha={})
    dvo.OPS.append(op)
    dvo.CUSTOM_DVE_SPECS[name] = spec
    dvo._SUB_OPCODE_FOR_NAME[name] = dvo._CUSTOM_DVE_ROW_BASE + len(dvo.OPS) - 1
    assert dvo._SUB_OPCODE_FOR_NAME[name] < 0x20
    for ver in ("v3", "v4"):
        sha = DveOpSpec(
            name=name,
            opcode=dvo.get_dve_sub_opcode(name),
            uops=lower(spec, ver=ver),
            rd1_en=True,
        ).sha(ver)
        op.uops_sha[ver] = sha
    return op


_absd = Bin(AluOp.ABSOLUTE_DIFF, Src0, Src1)
# accum_out[p] = s1 + sum_j relu(|in0-in1|)^2 - relu(|in0-in1| - s0)^2
# (AP seeding of the accumulator is broken -> literal 0.0, one column per call)
SL1_FUSED = _register(
    "SL1_FUSED_ANT",
    Spec(body=sq(relu(_absd)) - sq(relu(_absd - C0)), accum=_op_add,
         accum_init=C1, reference=_ref_sl1),
)
# accum_out[p] = s0 + sum_j (1 - in0)^2 * in1
FOCAL_TAIL = _register(
    "FOCAL_TAIL_ANT",
    Spec(body=sq(One - Src0) * Src1, accum=_op_add, accum_init=C0,
         reference=_ref_ft),
)

# ---------------------------------------------------------------------------
# device kernel (SPMD; one batch element per core)
# ---------------------------------------------------------------------------

_NC_CACHE = None


def build_kernel():
    global _NC_CACHE
    if _NC_CACHE is not None:
        return _NC_CACHE
    nc = bacc.Bacc()

    cls_in = nc.dram_tensor("cls_in", [A, C, HW], F32, kind="ExternalInput")
    box_in = nc.dram_tensor("box_in", [A, C, HW], F32, kind="ExternalInput")
    hot_in = nc.dram_tensor("hot_in", [PARTS, FJ], BF16, kind="ExternalInput")
    alf_in = nc.dram_tensor("alf_in", [PARTS, FJ], BF16, kind="ExternalInput")
    wd_in = nc.dram_tensor("wd_in", [PARTS, G], BF16, kind="ExternalInput")
    wnan_in = nc.dram_tensor("wnan_in", [PARTS, C, FREE], F32, kind="ExternalInput")
    out_cls = nc.dram_tensor("out_cls", [PARTS, len(GROUPS)], F32, kind="ExternalOutput")
    out_box = nc.dram_tensor("out_box", [PARTS, A], F32, kind="ExternalOutput")

    EXP = mybir.ActivationFunctionType.Exp
    LN = mybir.ActivationFunctionType.Ln

    with tile.TileContext(nc) as tc:
        with (
            tc.tile_pool(name="consts", bufs=1) as consts,
            tc.tile_pool(name="loads", bufs=3) as loads,
            tc.tile_pool(name="work", bufs=3) as work,
            tc.tile_pool(name="grp", bufs=2) as grp,
            tc.tile_pool(name="psum", bufs=1, space="PSUM") as psum,
            tc.tile_pool(name="outs", bufs=1) as outs,
        ):
            hot_t = consts.tile([PARTS, FJ], BF16)
            nc.sync.dma_start(out=hot_t, in_=hot_in.ap())
            alf_t = consts.tile([PARTS, FJ], BF16)
            nc.sync.dma_start(out=alf_t, in_=alf_in.ap())
            wd_t = consts.tile([PARTS, G], BF16)
            nc.sync.dma_start(out=wd_t, in_=wd_in.ap())
            wnan_t = consts.tile([PARTS, C, FREE], F32)
            nc.sync.dma_start(out=wnan_t, in_=wnan_in.ap())

            acc_cls = outs.tile([PARTS, len(GROUPS)], F32)
            nc.vector.memset(acc_cls, 0.0)
            acc_box = outs.tile([PARTS, A], F32)

            for gi, (g0, g1) in enumerate(GROUPS):
                na = g1 - g0
                gp = 32 * na  # partitions used in this group's PSUM tiles
                s_ps = psum.tile([PARTS, FJ], F32, tag="s_ps")
                et_ps = psum.tile([PARTS, FJ], F32, tag="et_ps")
                for r in range(na):
                    a = g0 + r
                    # ---- cls: exp, one-hot mult, PE class-sums ----
                    x_t = loads.tile([PARTS, FJ], F32, tag="x_t")
                    for c in range(C):
                        nc.sync.dma_start(
                            out=x_t[32 * c : 32 * (c + 1), :],
                            in_=cls_in.ap()[a, c].rearrange("(g j) -> g j", g=G),
                        )
                    e_t = work.tile([PARTS, FJ], BF16, tag="e_t")
                    nc.scalar.activation(e_t, x_t, EXP)
                    he_t = work.tile([PARTS, FJ], BF16, tag="he_t")
                    nc.vector.tensor_mul(he_t, e_t, hot_t)
                    for ch in range(4):
                        sl = slice(512 * ch, 512 * (ch + 1))
                        nc.tensor.matmul(
                            s_ps[32 * r : 32 * r + 32, sl], wd_t,
                            e_t[:, sl], start=True, stop=True,
                            tile_position=(0, 32 * r),
                        )
                        nc.tensor.matmul(
                            et_ps[32 * r : 32 * r + 32, sl], wd_t,
                            he_t[:, sl], start=True, stop=True,
                            tile_position=(0, 32 * r),
                        )

                    # ---- box (independent; pixel layout) ----
                    p_t = loads.tile([PARTS, C, FREE], F32, tag="p_t")
                    nc.sync.dma_start(
                        out=p_t,
                        in_=box_in.ap()[a].rearrange("c (k j) -> k c j", j=FREE),
                    )
                    sl_junk = work.tile([PARTS, C, FREE], BF16, tag="sl_junk")
                    nc.vector._custom_dve(
                        SL1_FUSED, out=sl_junk, in0=p_t, in1=wnan_t,
                        s0=1.0, s1=0.0, accum_out=acc_box[:, a : a + 1],
                    )

                # ---- per-group cls tail ----
                logs_t = grp.tile([PARTS, FJ], BF16, tag="logs_t")
                nc.scalar.activation(logs_t[:gp], s_ps[:gp], LN)
                loget_t = grp.tile([PARTS, FJ], BF16, tag="loget_t")
                nc.scalar.activation(loget_t[:gp], et_ps[:gp], LN)
                ce_t = grp.tile([PARTS, FJ], BF16, tag="ce_t")
                nc.vector.tensor_sub(ce_t[:gp], logs_t[:gp], loget_t[:gp])
                pt_t = grp.tile([PARTS, FJ], BF16, tag="pt_t")
                nc.scalar.activation(pt_t[:gp], ce_t[:gp], EXP, scale=-1.0)
                ace_t = grp.tile([PARTS, FJ], BF16, tag="ace_t")
                nc.vector.tensor_mul(ace_t[:gp], alf_t[:gp], ce_t[:gp])
                ft_junk = grp.tile([PARTS, FJ], BF16, tag="ft_junk")
                nc.vector._custom_dve(
                    FOCAL_TAIL, out=ft_junk[:gp], in0=pt_t[:gp], in1=ace_t[:gp],
                    s0=0.0, s1=0.0, accum_out=acc_cls[:gp, gi : gi + 1],
                )

            nc.sync.dma_start(out=out_cls.ap(), in_=acc_cls)
            nc.sync.dma_start(out=out_box.ap(), in_=acc_box)

    # The act-table-load pass picks the FIRST set containing each function,
    # thrashing between exp_and_others and natural_log (25 x 1.28us reloads).
    # Restrict eligibility to the one set containing Exp AND Ln (index
    # positions preserved -- act_func_set_id is the insertion-order index).
    _orig_gat = bacc.get_activation_tables
    _COMBINED = "natural_log_exp_and_others"

    def _patched_gat(arch):
        t = _orig_gat(arch)
        return {name: (fns if name == _COMBINED else set()) for name, fns in t.items()}

    bacc.get_activation_tables = _patched_gat
    try:
        nc.finalize()
    finally:
        bacc.get_activation_tables = _orig_gat
    _NC_CACHE = nc
    return nc


# ---------------------------------------------------------------------------
# host side
# ---------------------------------------------------------------------------


def _rasterize_np(boxes, labels):
    """Exact numpy port of the reference _rasterize (truncation, clipping,
    last-covering-box-wins)."""
    Bn, Nn = labels.shape
    bi = boxes.astype(np.int32)
    x1 = np.clip(bi[..., 0], 0, W - 1)
    y1 = np.clip(bi[..., 1], 0, H - 1)
    x2 = np.clip(bi[..., 2], 0, W - 1)
    y2 = np.clip(bi[..., 3], 0, H - 1)
    ys = np.arange(H)
    xs = np.arange(W)
    inside = (
        (ys[None, None, :, None] >= y1[:, :, None, None])
        & (ys[None, None, :, None] <= y2[:, :, None, None])
        & (xs[None, None, None, :] >= x1[:, :, None, None])
        & (xs[None, None, None, :] <= x2[:, :, None, None])
    )  # (B,N,H,W)
    box_ids = np.arange(Nn, dtype=np.int32)[None, :, None, None]
    last = np.max(np.where(inside, box_ids, -1), axis=1)  # (B,H,W)
    valid = last >= 0
    idx = np.maximum(last, 0)
    bsel = np.arange(Bn)[:, None, None]
    tgt_label = np.where(valid, labels[bsel, idx], 0)  # (B,H,W)
    tgt_box = boxes[bsel, idx]  # (B,H,W,4)
    return tgt_label, tgt_box, valid


_LAST_RESULT = None  # BassKernelResults of the last run (for profiling)

# delta weights: wd[(c,g), g'] = 1 iff g == g' -- same for every core
_WDELTA = np.tile(np.eye(G, dtype=ml_dtypes.bfloat16), (C, 1))  # [128, 32]


def kernel(cls_scores, bbox_preds, boxes, labels, alpha):
    global _LAST_RESULT
    cls_scores = np.ascontiguousarray(cls_scores, dtype=np.float32)
    bbox_preds = np.ascontiguousarray(bbox_preds, dtype=np.float32)
    boxes = np.asarray(boxes, dtype=np.float32)
    labels = np.asarray(labels, dtype=np.int32)
    alpha = np.asarray(alpha, dtype=np.float32)

    tgt_label, tgt_box, valid = _rasterize_np(boxes, labels)

    in_maps = []
    for b in range(B):
        t = tgt_label[b].reshape(HW)  # int, [65536]
        v = valid[b].reshape(HW)
        # cls layout: partition (c,g), pixel = g*2048 + j
        tg = t.reshape(G, FJ)
        hot = (tg[None, :, :] == np.arange(C)[:, None, None]).reshape(PARTS, FJ)
        alf = np.tile(alpha[tg], (C, 1))  # [128, 2048]
        # box layout: partition k, pixel = k*512 + j
        tb = tgt_box[b].reshape(HW, 4).T  # [4, 65536] float32
        wn = np.where(v[None, :], tb, np.nan).astype(np.float32)
        wn = wn.reshape(C, PARTS, FREE).transpose(1, 0, 2).copy()  # [128,4,512]
        in_maps.append(
            {
                "cls_in": cls_scores[b].reshape(A, C, HW),
                "box_in": bbox_preds[b].reshape(A, C, HW),
                "hot_in": hot.astype(ml_dtypes.bfloat16),
                "alf_in": alf.astype(ml_dtypes.bfloat16),
                "wd_in": _WDELTA,
                "wnan_in": wn,
            }
        )

    nc = build_kernel()
    res = run_bass_kernel_spmd(nc, in_maps, core_ids=list(range(B)))
    _LAST_RESULT = res

    cls_loss_b = np.empty(B, np.float64)
    box_loss_b = np.empty(B, np.float64)
    for b in range(B):
        cls_sum = float(res.results[b]["out_cls"].astype(np.float64).sum())
        box_sum = float(res.results[b]["out_box"].astype(np.float64).sum()) * 0.5
        cls_loss_b[b] = cls_sum / (A * HW)
        cnt = float(valid[b].sum()) * (A * 4)
        box_loss_b[b] = box_sum / max(cnt, 1.0) if cnt > 0 else 0.0

    cls_loss = np.float32(cls_loss_b.mean())
    box_loss = np.float32(box_loss_b.mean())
    total = np.float32(cls_loss + box_loss)
    return total, cls_loss, box_loss


# revision 7
# speedup vs baseline: 1.1143x; 1.1143x over previous
"""DetectionLoss Trainium2 kernel (8 NeuronCores, pure data parallel over B).

Reference computation (see problem statement):
  - rasterize N=16 boxes per batch to per-pixel target label / target box /
    valid mask (host, numpy -- tiny work on tiny inputs)
  - focal classification loss over cls_scores (B,A,C,H,W) = (8,9,4,256,256)
  - masked SmoothL1 box loss over bbox_preds  (B,A,4,H,W)
  - scalar means -> (total, cls_loss, box_loss)

Device strategy (one batch element per core), pixel-on-partition layout
(pixel = k*512 + j, k = partition):

cls, per anchor a:
  e = exp(x)            ACT, f32 -> bf16, [128, 4, 512]
  he = e * onehot       DVE TT bf16 2x
  S  = sum_c e          GPSIMD adds (offload -- DVE is the bottleneck)
  Et = sum_c he         DVE adds bf16 2x
  logS = ln(S)          ACT      (logEt == target logit exactly: the one-hot
  logEt = ln(Et)        ACT       mult zeroes other classes, sum adds zeros)
  ce = logS - logEt     GPSIMD
  pt = exp(-ce)         ACT
  ace = alpha_t * ce    DVE TT bf16
  cls accum             custom DVE op: sum (1-pt)^2 * ace

box, per anchor: one fused custom DVE op:
  accum += sum relu(|p-w|)^2 - relu(|p-w|-1)^2 == 2 * smoothl1(p-t) * valid
  with w = target-box-or-NaN; relu(NaN)=0 on DVE scrubs invalid pixels.

host: final scalar reductions over the tiny per-partition accumulators.

The act-table pass is patched to use the single table set containing both
Exp and Ln (otherwise it reloads tables 25x for ~32us).
"""

import os
import sys

sys.path.insert(0, "/opt/trn_rl_repo")

from operator import add as _op_add

import ml_dtypes
import numpy as np

import concourse.bacc as bacc
import concourse.tile as tile
from concourse import mybir
from concourse.bass_utils import run_bass_kernel_spmd
from concourse.dve_spec import AluOp, Bin, C0, C1, One, Spec, Src0, Src1, lower, relu, sq
from concourse.dve_uop import DveOpSpec
import concourse.dve_ops as dvo

BF16 = mybir.dt.bfloat16
F32 = mybir.dt.float32

GAMMA = 2.0
B, A, C, H, W, N = 8, 9, 4, 256, 256, 16
HW = H * W  # 65536
PARTS = 128
FREE = HW // PARTS  # 512

# ---------------------------------------------------------------------------
# custom DVE ops
# ---------------------------------------------------------------------------


def _dve_relu(x):
    # DVE MAX semantics: max(NaN, 0) = 0 (numpy max propagates NaN)
    return np.maximum(np.nan_to_num(x, nan=0.0, posinf=np.inf, neginf=-np.inf), 0)


def _as_col(v, P):
    a = np.asarray(v, np.float32)
    return a.reshape(-1, 1) if a.ndim else np.full((P, 1), float(a), np.float32)


def _ref_sl1(in0, in1, s0, s1, imm2):
    P = in0.shape[0]
    a = np.abs(in0.astype(np.float32) - in1.astype(np.float32))
    body = _dve_relu(a) ** 2 - _dve_relu(a - _as_col(s0, P)) ** 2
    acc = _as_col(s1, P) + body.reshape(P, -1).sum(axis=-1, keepdims=True)
    return body.astype(np.float32), acc


def _ref_ft(in0, in1, s0, s1, imm2):
    P = in0.shape[0]
    body = (1.0 - in0.astype(np.float32)) ** 2 * in1.astype(np.float32)
    acc = _as_col(s0, P) + body.reshape(P, -1).sum(axis=-1, keepdims=True)
    return body.astype(np.float32), acc


def _register(name, spec):
    for op in dvo.OPS:
        if op.name == name:  # idempotent across re-imports
            return op
    op = dvo.DveOp(name, spec, subdim=False, uops_sha={})
    dvo.OPS.append(op)
    dvo.CUSTOM_DVE_SPECS[name] = spec
    dvo._SUB_OPCODE_FOR_NAME[name] = dvo._CUSTOM_DVE_ROW_BASE + len(dvo.OPS) - 1
    assert dvo._SUB_OPCODE_FOR_NAME[name] < 0x20
    for ver in ("v3", "v4"):
        sha = DveOpSpec(
            name=name,
            opcode=dvo.get_dve_sub_opcode(name),
            uops=lower(spec, ver=ver),
            rd1_en=True,
        ).sha(ver)
        op.uops_sha[ver] = sha
    return op


_absd = Bin(AluOp.ABSOLUTE_DIFF, Src0, Src1)
# accum_out[p] = s1 + sum_j relu(|in0-in1|)^2 - relu(|in0-in1| - s0)^2
# (AP seeding of the accumulator is broken -> literal 0.0, one column per call)
SL1_FUSED = _register(
    "SL1_FUSED_ANT",
    Spec(body=sq(relu(_absd)) - sq(relu(_absd - C0)), accum=_op_add,
         accum_init=C1, reference=_ref_sl1),
)
# accum_out[p] = s0 + sum_j (1 - in0)^2 * in1
FOCAL_TAIL = _register(
    "FOCAL_TAIL_ANT",
    Spec(body=sq(One - Src0) * Src1, accum=_op_add, accum_init=C0,
         reference=_ref_ft),
)

# ---------------------------------------------------------------------------
# device kernel (SPMD; one batch element per core)
# ---------------------------------------------------------------------------

_NC_CACHE = None


def build_kernel():
    global _NC_CACHE
    if _NC_CACHE is not None:
        return _NC_CACHE
    nc = bacc.Bacc()

    cls_in = nc.dram_tensor("cls_in", [A, C, HW], F32, kind="ExternalInput")
    box_in = nc.dram_tensor("box_in", [A, C, HW], F32, kind="ExternalInput")
    hot_in = nc.dram_tensor("hot_in", [PARTS, C, FREE], BF16, kind="ExternalInput")
    alf_in = nc.dram_tensor("alf_in", [PARTS, FREE], BF16, kind="ExternalInput")
    wnan_in = nc.dram_tensor("wnan_in", [PARTS, C, FREE], F32, kind="ExternalInput")
    out_cls = nc.dram_tensor("out_cls", [PARTS, A], F32, kind="ExternalOutput")
    out_box = nc.dram_tensor("out_box", [PARTS, A], F32, kind="ExternalOutput")

    EXP = mybir.ActivationFunctionType.Exp
    LN = mybir.ActivationFunctionType.Ln

    with tile.TileContext(nc) as tc:
        with (
            tc.tile_pool(name="consts", bufs=1) as consts,
            tc.tile_pool(name="loads", bufs=4) as loads,
            tc.tile_pool(name="work", bufs=3) as work,
            tc.tile_pool(name="small", bufs=3) as small,
            tc.tile_pool(name="outs", bufs=1) as outs,
        ):
            hot_t = consts.tile([PARTS, C, FREE], BF16)
            nc.sync.dma_start(out=hot_t, in_=hot_in.ap())
            alf_t = consts.tile([PARTS, FREE], BF16)
            nc.sync.dma_start(out=alf_t, in_=alf_in.ap())
            wnan_t = consts.tile([PARTS, C, FREE], F32)
            nc.sync.dma_start(out=wnan_t, in_=wnan_in.ap())

            acc_cls = outs.tile([PARTS, A], F32)
            acc_box = outs.tile([PARTS, A], F32)

            for a in range(A):
                # ---------------- classification ----------------
                x_t = loads.tile([PARTS, C, FREE], F32, tag="x_t")
                nc.sync.dma_start(
                    out=x_t,
                    in_=cls_in.ap()[a].rearrange("c (k j) -> k c j", j=FREE),
                )
                e_t = work.tile([PARTS, C, FREE], BF16, tag="e_t")
                nc.scalar.activation(e_t, x_t, EXP)
                he_t = work.tile([PARTS, C, FREE], BF16, tag="he_t")
                nc.vector.tensor_mul(he_t, e_t, hot_t)

                # S = sum_c e on GPSIMD (DVE is the critical engine)
                s01 = small.tile([PARTS, FREE], BF16, tag="s01")
                s23 = small.tile([PARTS, FREE], BF16, tag="s23")
                s_t = small.tile([PARTS, FREE], BF16, tag="s_t")
                nc.gpsimd.tensor_add(s01, e_t[:, 0, :], e_t[:, 1, :])
                nc.gpsimd.tensor_add(s23, e_t[:, 2, :], e_t[:, 3, :])
                nc.gpsimd.tensor_add(s_t, s01, s23)

                # Et = sum_c he on DVE (bf16 2x)
                t01 = small.tile([PARTS, FREE], BF16, tag="t01")
                t23 = small.tile([PARTS, FREE], BF16, tag="t23")
                et_t = small.tile([PARTS, FREE], BF16, tag="et_t")
                nc.vector.tensor_add(t01, he_t[:, 0, :], he_t[:, 1, :])
                nc.vector.tensor_add(t23, he_t[:, 2, :], he_t[:, 3, :])
                nc.vector.tensor_add(et_t, t01, t23)

                logs_t = small.tile([PARTS, FREE], BF16, tag="logs_t")
                nc.scalar.activation(logs_t, s_t, LN)
                loget_t = small.tile([PARTS, FREE], BF16, tag="loget_t")
                nc.scalar.activation(loget_t, et_t, LN)

                ce_t = small.tile([PARTS, FREE], BF16, tag="ce_t")
                nc.gpsimd.tensor_sub(ce_t, logs_t, loget_t)
                pt_t = small.tile([PARTS, FREE], BF16, tag="pt_t")
                nc.scalar.activation(pt_t, ce_t, EXP, scale=-1.0)
                ace_t = small.tile([PARTS, FREE], BF16, tag="ace_t")
                nc.vector.tensor_mul(ace_t, alf_t, ce_t)

                ft_junk = small.tile([PARTS, FREE], BF16, tag="ft_junk")
                nc.vector._custom_dve(
                    FOCAL_TAIL, out=ft_junk, in0=pt_t, in1=ace_t,
                    s0=0.0, s1=0.0, accum_out=acc_cls[:, a : a + 1],
                )

                # ---------------- box ----------------
                p_t = loads.tile([PARTS, C, FREE], F32, tag="p_t")
                nc.sync.dma_start(
                    out=p_t,
                    in_=box_in.ap()[a].rearrange("c (k j) -> k c j", j=FREE),
                )
                sl_junk = work.tile([PARTS, C, FREE], BF16, tag="sl_junk")
                nc.vector._custom_dve(
                    SL1_FUSED, out=sl_junk, in0=p_t, in1=wnan_t,
                    s0=1.0, s1=0.0, accum_out=acc_box[:, a : a + 1],
                )

            nc.sync.dma_start(out=out_cls.ap(), in_=acc_cls)
            nc.sync.dma_start(out=out_box.ap(), in_=acc_box)

    # The act-table-load pass picks the FIRST set containing each function,
    # thrashing between exp_and_others and natural_log (25 x 1.28us reloads).
    # Restrict eligibility to the one set containing Exp AND Ln (index
    # positions preserved -- act_func_set_id is the insertion-order index).
    _orig_gat = bacc.get_activation_tables
    _COMBINED = "natural_log_exp_and_others"

    def _patched_gat(arch):
        t = _orig_gat(arch)
        return {name: (fns if name == _COMBINED else set()) for name, fns in t.items()}

    bacc.get_activation_tables = _patched_gat
    try:
        nc.finalize()
    finally:
        bacc.get_activation_tables = _orig_gat
    _NC_CACHE = nc
    return nc


# ---------------------------------------------------------------------------
# host side
# ---------------------------------------------------------------------------


def _rasterize_np(boxes, labels):
    """Exact numpy port of the reference _rasterize (truncation, clipping,
    last-covering-box-wins)."""
    Bn, Nn = labels.shape
    bi = boxes.astype(np.int32)
    x1 = np.clip(bi[..., 0], 0, W - 1)
    y1 = np.clip(bi[..., 1], 0, H - 1)
    x2 = np.clip(bi[..., 2], 0, W - 1)
    y2 = np.clip(bi[..., 3], 0, H - 1)
    ys = np.arange(H)
    xs = np.arange(W)
    inside = (
        (ys[None, None, :, None] >= y1[:, :, None, None])
        & (ys[None, None, :, None] <= y2[:, :, None, None])
        & (xs[None, None, None, :] >= x1[:, :, None, None])
        & (xs[None, None, None, :] <= x2[:, :, None, None])
    )  # (B,N,H,W)
    box_ids = np.arange(Nn, dtype=np.int32)[None, :, None, None]
    last = np.max(np.where(inside, box_ids, -1), axis=1)  # (B,H,W)
    valid = last >= 0
    idx = np.maximum(last, 0)
    bsel = np.arange(Bn)[:, None, None]
    tgt_label = np.where(valid, labels[bsel, idx], 0)  # (B,H,W)
    tgt_box = boxes[bsel, idx]  # (B,H,W,4)
    return tgt_label, tgt_box, valid


_LAST_RESULT = None  # BassKernelResults of the last run (for profiling)


def kernel(cls_scores, bbox_preds, boxes, labels, alpha):
    global _LAST_RESULT
    cls_scores = np.ascontiguousarray(cls_scores, dtype=np.float32)
    bbox_preds = np.ascontiguousarray(bbox_preds, dtype=np.float32)
    boxes = np.asarray(boxes, dtype=np.float32)
    labels = np.asarray(labels, dtype=np.int32)
    alpha = np.asarray(alpha, dtype=np.float32)

    tgt_label, tgt_box, valid = _rasterize_np(boxes, labels)

    in_maps = []
    for b in range(B):
        t = tgt_label[b].reshape(HW)  # int, [65536]
        v = valid[b].reshape(HW)
        tk = t.reshape(PARTS, FREE)
        hot = (tk[:, None, :] == np.arange(C)[None, :, None]).astype(
            ml_dtypes.bfloat16
        )  # [128, 4, 512]
        alf = alpha[tk].astype(ml_dtypes.bfloat16)  # [128, 512]
        tb = tgt_box[b].reshape(HW, 4).T  # [4, 65536] float32
        wn = np.where(v[None, :], tb, np.nan).astype(np.float32)
        wn = wn.reshape(C, PARTS, FREE).transpose(1, 0, 2).copy()  # [128,4,512]
        in_maps.append(
            {
                "cls_in": cls_scores[b].reshape(A, C, HW),
                "box_in": bbox_preds[b].reshape(A, C, HW),
                "hot_in": hot,
                "alf_in": alf,
                "wnan_in": wn,
            }
        )

    nc = build_kernel()
    res = run_bass_kernel_spmd(nc, in_maps, core_ids=list(range(B)))
    _LAST_RESULT = res

    cls_loss_b = np.empty(B, np.float64)
    box_loss_b = np.empty(B, np.float64)
    for b in range(B):
        cls_sum = float(res.results[b]["out_cls"].astype(np.float64).sum())
        box_sum = float(res.results[b]["out_box"].astype(np.float64).sum()) * 0.5
        cls_loss_b[b] = cls_sum / (A * HW)
        cnt = float(valid[b].sum()) * (A * 4)
        box_loss_b[b] = box_sum / max(cnt, 1.0) if cnt > 0 else 0.0

    cls_loss = np.float32(cls_loss_b.mean())
    box_loss = np.float32(box_loss_b.mean())
    total = np.float32(cls_loss + box_loss)
    return total, cls_loss, box_loss
